# revision 13
# baseline (speedup 1.0000x reference)
"""Trainium2 Bass kernel for nn_DecoderCell (LFADS-style decoder cell).

Strategy: pure data parallel over 8 NeuronCores (batch 32768 -> 4096/core).
v2: bf16 end-to-end.
 - Host packs inputs feature-major as ONE bf16 tensor per core
   ([hg; hc; fa; ci] = 1152 rows) so each batch tile needs a single big
   input DMA; eps/xext ride separate small tensors. Output is staged in
   one bf16 SBUF tile per batch tile (compute ops write final values
   directly into their outT row positions) and stored with one DMA.
 - All matmuls bf16 x bf16 -> f32 PSUM (1 cyc/row, same rate as f32r,
   half the DMA bytes).
 - Sigmoid is computed as 0.5 + 0.5*tanh(x/2) so every activation
   (tanh/exp/identity/copy) lives in ONE act table set -> zero act-table
   switches. The 0.5 factors fold into tensor_scalar ops / W_n weights.
 - GRU update h' = n + a*(h-n) with a = 0.5 + 0.5*t_z via DVE
   tensor_scalar (4x bf16 mode) + tensor_tensor (2x), split across
   Vector and GPSIMD engines.
"""

import sys

sys.path.insert(0, "/opt/trn_rl_repo")

import numpy as np

import concourse.bacc as bacc
import concourse.tile as tile
import concourse.mybir as mybir

N_CORES = 8
BATCH = 32768
B_CORE = BATCH // N_CORES  # 4096
NT = 512                   # batch tile (free dim per matmul / PSUM bank)
NTILES = B_CORE // NT      # 8

GEN, CON, CO, FAC, CIE, EXT = 512, 256, 64, 128, 128, 16
X_DIM = 2 * CIE + EXT      # 272
H_DIM = GEN + CON + 3 * CO + EXT + FAC  # 1104
CLIP = 5.0

IN_ROWS = GEN + CON + FAC + 2 * CIE     # 1152 = 9 chunks of 128
IN_CH = IN_ROWS // 128                  # 9
OUT_CH = 8                              # stg: hg2,hc2,mean/std,gi+xext

F32 = mybir.dt.float32
BF16 = mybir.dt.bfloat16
FP8 = mybir.dt.float8e4
DRM = mybir.MatmulPerfMode.DoubleRow
WGN_SCALE = 32.0
AF = mybir.ActivationFunctionType
ALU = mybir.AluOpType

KNOBS = {
    "in_bufs": 3, "stg_bufs": 3, "eps_bufs": 2,
    "act_bufs": 2, "tmp_bufs": 2,
    "p2_bufs": 2, "p1_bufs": 1,
    # engine assignment: True -> gpsimd(Pool), False -> vector(DVE)
    "gp_rh_c": False, "gp_d_c": False,
    "gp_rh_g": False, "gp_d_g": False,
    # fp8 DoubleRow matmuls (precision-validated against the reference)
    "fp8_gen_h": True, "fp8_gen_rh": True, "fp8_con_rh": False,
    "fp8_con_h": True,
    "h8_from_host": True,
    "fc_on_act": True, "f_before_e": False,
}


def ts(i, s):
    return slice(i * s, (i + 1) * s)


def build_program(has_bias: bool, repeat: int = 1):
    nc = bacc.Bacc("TRN2", target_bir_lowering=False, debug=False,
                   num_devices=N_CORES)

    inT = nc.dram_tensor("inT", [IN_ROWS, B_CORE], BF16, kind="ExternalInput")
    xextT = nc.dram_tensor("xextT", [EXT, B_CORE], BF16, kind="ExternalInput")
    epsT = nc.dram_tensor("epsT", [CO, B_CORE], BF16, kind="ExternalInput")
    Wci_d = nc.dram_tensor("Wci", [384, 768], BF16, kind="ExternalInput")
    Wczr_d = nc.dram_tensor("Wczr", [256, 512],
                            FP8 if KNOBS["fp8_con_h"] else BF16,
                            kind="ExternalInput")
    Wcn_d = nc.dram_tensor("Wcn", [256, 256],
                           FP8 if KNOBS["fp8_con_rh"] else BF16,
                           kind="ExternalInput")
    Wco_d = nc.dram_tensor("Wco", [256, 128], BF16, kind="ExternalInput")
    Wgi_d = nc.dram_tensor("Wgi", [80, 1536], BF16, kind="ExternalInput")
    Wgzr_d = nc.dram_tensor("Wgzr", [512, 1024],
                            FP8 if KNOBS["fp8_gen_h"] else BF16,
                            kind="ExternalInput")
    Wgn_d = nc.dram_tensor("Wgn", [512, 512],
                           FP8 if KNOBS["fp8_gen_rh"] else BF16,
                           kind="ExternalInput")
    Wfac_d = nc.dram_tensor("Wfac", [512, 128], BF16, kind="ExternalInput")
    H8_ROWS = (GEN if KNOBS["fp8_gen_h"] else 0) + \
        (CON if KNOBS["fp8_con_h"] else 0)
    if H8_ROWS and KNOBS["h8_from_host"]:
        h8T = nc.dram_tensor("h8T", [H8_ROWS, B_CORE], FP8,
                             kind="ExternalInput")
    if has_bias:
        # per-partition bias columns, pre-scaled on host where needed
        Bczr_d = nc.dram_tensor("Bczr", [128, 4], F32, kind="ExternalInput")
        Bcn_d = nc.dram_tensor("Bcn", [128, 2], F32, kind="ExternalInput")
        Bgzr_d = nc.dram_tensor("Bgzr", [128, 8], F32, kind="ExternalInput")
        Bgn_d = nc.dram_tensor("Bgn", [128, 4], F32, kind="ExternalInput")
        Bco_d = nc.dram_tensor("Bco", [128, 1], F32, kind="ExternalInput")

    outT = nc.dram_tensor("outT", [H_DIM, B_CORE], BF16,
                          kind="ExternalOutput")

    # staging row map: hg2 rows 0..511 stg[:,0:4]; hc2 rows 512..767
    # stg[:,4:6]; mean rows 768..831 stg[0:64,6]; std rows 832..895
    # stg[64:128,6] (filled by SB->SB DMA from a base-0 tile, since all
    # TensorTensor math must be partition-base aligned); gi rows 896..975
    # stg[0:80,7] ([gi_co(64); xext(16)]). fc (rows 976..1103) bypasses
    # staging and is stored from its own base-0 tile.

    with tile.TileContext(nc) as tc:
        with (
            tc.tile_pool(name="w", bufs=1) as wp,
            tc.tile_pool(name="io", bufs=KNOBS["in_bufs"]) as io,
            tc.tile_pool(name="stg", bufs=KNOBS["stg_bufs"]) as stgp,
            tc.tile_pool(name="eps", bufs=KNOBS["eps_bufs"]) as epsp,
            tc.tile_pool(name="act", bufs=KNOBS["act_bufs"]) as act,
            tc.tile_pool(name="tmp", bufs=KNOBS["tmp_bufs"]) as tmp,
            tc.tile_pool(name="p2", bufs=KNOBS["p2_bufs"], space="PSUM") as p2,
            tc.tile_pool(name="p1", bufs=KNOBS["p1_bufs"], space="PSUM") as p1,
        ):
            # ---- load weights once (feature-major [k, m]) ----
            def wload(dram, K, M, tag):
                kc = (K + 127) // 128
                dt = dram.dtype
                if K % 128 == 0:
                    t = wp.tile([128, kc, M], dt, tag=tag, name=tag)
                    nc.sync.dma_start(
                        out=t,
                        in_=dram[:, :].rearrange("(c p) m -> p c m", p=128))
                else:
                    assert kc == 1
                    t = wp.tile([K, 1, M], dt, tag=tag, name=tag)
                    nc.sync.dma_start(out=t, in_=dram[:, :].unsqueeze(1))
                return t

            Wci = wload(Wci_d, 384, 768, "Wci")
            Wczr = wload(Wczr_d, 256, 512, "Wczr")
            Wcn = wload(Wcn_d, 256, 256, "Wcn")
            Wco = wload(Wco_d, 256, 128, "Wco")
            Wgi = wload(Wgi_d, 80, 1536, "Wgi")
            Wgzr = wload(Wgzr_d, 512, 1024, "Wgzr")
            Wgn = wload(Wgn_d, 512, 512, "Wgn")
            Wfac = wload(Wfac_d, 512, 128, "Wfac")
            if has_bias:
                Bczr = wp.tile([128, 4], F32, tag="Bczr", name="Bczr")
                nc.sync.dma_start(out=Bczr, in_=Bczr_d[:, :])
                Bcn = wp.tile([128, 2], F32, tag="Bcn", name="Bcn")
                nc.sync.dma_start(out=Bcn, in_=Bcn_d[:, :])
                Bgzr = wp.tile([128, 8], F32, tag="Bgzr", name="Bgzr")
                nc.sync.dma_start(out=Bgzr, in_=Bgzr_d[:, :])
                Bgn = wp.tile([128, 4], F32, tag="Bgn", name="Bgn")
                nc.sync.dma_start(out=Bgn, in_=Bgn_d[:, :])
                Bco = wp.tile([128, 1], F32, tag="Bco", name="Bco")
                nc.sync.dma_start(out=Bco, in_=Bco_d[:, :])

            def mm_group(psum_out, pairs):
                n = len(pairs)
                for i, p in enumerate(pairs):
                    lh, rh = p[0], p[1]
                    pm_ = DRM if len(p) > 2 else None
                    nc.tensor.matmul(psum_out, lh, rh,
                                     start=(i == 0), stop=(i == n - 1),
                                     perf_mode=pm_)

            if not has_bias:
                Bczr = Bcn = Bgzr = Bgn = Bco = None

            def activ(dst, src, func, scale=1.0, bias_tile=None, bias_c=0,
                      nch=1):
                if bias_tile is not None:
                    for c in range(nch):
                        nc.scalar.activation(
                            dst[:, c, :] if nch > 1 else dst,
                            src[:, c, :] if nch > 1 else src,
                            func, scale=scale,
                            bias=bias_tile[:, bias_c + c:bias_c + c + 1])
                else:
                    nc.scalar.activation(dst, src, func, scale=scale)

            def eng(gp):
                return nc.gpsimd if gp else nc.vector

            # ------------- software-pipelined stages -------------
            # Per tile t: A=load+con-zr, B=con-n+h'c, C=co+gi,
            # D=gen-zr, E=gen-n+h'g, F=fac+store.
            # Steady emission: A[t+1] D[t] B[t+1] E[t] F[t-1] C[t+1]
            # keeps the in-order PE queue stall-free: every dependent
            # stage has another tile's matmuls emitted between it and
            # its producer.
            S = {}

            def ev(gp):
                return nc.gpsimd if gp else nc.vector

            def stA(t):
                j = t % NTILES
                jc = ts(j, NT)
                s = S[t] = {}
                tin = io.tile([128, IN_CH, NT], BF16, tag="tin", name="tin")
                nc.sync.dma_start(
                    out=tin,
                    in_=inT[:, jc].rearrange("(c p) n -> p c n", p=128))
                stg = stgp.tile([128, OUT_CH, NT], BF16, tag="stg",
                                name="stg")
                nc.sync.dma_start(out=stg[64:80, 7, :], in_=xextT[:, jc])
                ep = epsp.tile([64, NT], BF16, tag="ep", name="ep")
                nc.sync.dma_start(out=ep, in_=epsT[:, jc])
                s["tin"], s["stg"], s["ep"] = tin, stg, ep
                if H8_ROWS:
                    nch8 = H8_ROWS // 128
                    h8 = io.tile([128, nch8, NT], FP8, tag="h8", name="h8")
                    if KNOBS["h8_from_host"]:
                        nc.sync.dma_start(
                            out=h8,
                            in_=h8T[:, jc].rearrange("(c p) n -> p c n",
                                                     p=128))
                    else:
                        if KNOBS["fp8_gen_h"]:
                            nc.scalar.copy(h8[:, 0:4, :], tin[:, 0:4, :])
                        if KNOBS["fp8_con_h"]:
                            nc.scalar.copy(h8[:, nch8 - 2:nch8, :],
                                           tin[:, 4:6, :])
                    if KNOBS["fp8_gen_h"]:
                        s["hg8"] = h8[:, 0:4, :]
                    if KNOBS["fp8_con_h"]:
                        s["hc8"] = h8[:, nch8 - 2:nch8, :]
                hc = tin[:, 4:6, :]
                x_rhs = [tin[:, 7, :], tin[:, 8, :], tin[:, 6, :]]
                s["x_rhs"] = x_rhs

                tzr_c = act.tile([128, 4, NT], BF16, tag="tzr_c",
                                 name="tzr_c")
                for half in range(2):
                    pz = p2.tile([128, 2, NT], F32, tag="p2", name="pz_c")
                    for mi in range(2):
                        m = 2 * half + mi
                        pairs = [(Wci[:, k, ts(m, 128)], x_rhs[k])
                                 for k in range(3)]
                        if KNOBS["fp8_con_h"]:
                            pairs += [(Wczr[:, 0:2, ts(m, 128)],
                                       s["hc8"][:, 0:2, :], "dr")]
                        else:
                            pairs += [(Wczr[:, k, ts(m, 128)], hc[:, k, :])
                                      for k in range(2)]
                        mm_group(pz[:, mi, :], pairs)
                    activ(tzr_c[:, 2 * half:2 * half + 2, :], pz, AF.Tanh,
                          scale=0.5, bias_tile=Bczr, bias_c=2 * half, nch=2)
                s["tzr_c"] = tzr_c
                b_rc = tmp.tile([128, 2, NT], BF16, tag="b_rc", name="b_rc")
                nc.vector.tensor_scalar_add(b_rc, tzr_c[:, 2:4, :], 1.0)
                rh_c = tmp.tile([128, 2, NT],
                                FP8 if KNOBS["fp8_con_rh"] else BF16,
                                tag="rh_c", name="rh_c")
                ev(KNOBS["gp_rh_c"]).tensor_mul(rh_c, b_rc[:, :, :],
                                                hc[:, :, :])
                s["rh_c"] = rh_c

            def stB(t):
                s = S[t]
                tin, stg = s["tin"], s["stg"]
                hc, rh_c = tin[:, 4:6, :], s["rh_c"]
                pn = p2.tile([128, 2, NT], F32, tag="p2", name="pn_c")
                for mi in range(2):
                    pairs = [(Wci[:, k, ts(4 + mi, 128)], s["x_rhs"][k])
                             for k in range(3)]
                    if KNOBS["fp8_con_rh"]:
                        pairs += [(Wcn[:, 0:2, ts(mi, 128)],
                                   rh_c[:, 0:2, :], "dr")]
                    else:
                        pairs += [(Wcn[:, k, ts(mi, 128)], rh_c[:, k, :])
                                  for k in range(2)]
                    mm_group(pn[:, mi, :], pairs)
                n_c = act.tile([128, 2, NT], BF16, tag="n_c", name="n_c")
                activ(n_c, pn, AF.Tanh, bias_tile=Bcn, nch=2)
                a_zc = tmp.tile([128, 2, NT], BF16, tag="a_zc", name="a_zc")
                nc.vector.tensor_scalar(a_zc, s["tzr_c"][:, 0:2, :], 0.5, 0.5,
                                        op0=ALU.mult, op1=ALU.add)
                d_c = tmp.tile([128, 2, NT], BF16, tag="d_c", name="d_c")
                ev(KNOBS["gp_d_c"]).tensor_sub(d_c, hc, n_c[:, :, :])
                nc.vector.tensor_mul(d_c, a_zc[:, :, :], d_c[:, :, :])
                hc2 = stg[:, 4:6, :]
                nc.vector.tensor_add(hc2, n_c[:, :, :], d_c[:, :, :])
                nc.vector.tensor_scalar(hc2, hc2, -CLIP, CLIP,
                                        op0=ALU.max, op1=ALU.min)

            def stC(t):
                s = S[t]
                stg, ep = s["stg"], s["ep"]
                hc2 = stg[:, 4:6, :]
                pm = p1.tile([64, NT], F32, tag="pm", name="pm")
                mm_group(pm, [(Wco[:, k, 0:64], hc2[:, k, :])
                              for k in range(2)])
                pv = p1.tile([64, NT], F32, tag="pv", name="pv")
                mm_group(pv, [(Wco[:, k, 64:128], hc2[:, k, :])
                              for k in range(2)])
                mean = stg[0:64, 6, :]
                std_t = tmp.tile([64, NT], BF16, tag="std_t", name="std_t")
                if has_bias:
                    nc.scalar.activation(mean, pm, AF.Identity,
                                         bias=Bco[0:64, 0:1])
                    nc.scalar.activation(std_t, pv, AF.Exp,
                                         scale=0.5, bias=Bco[64:128, 0:1])
                else:
                    nc.scalar.copy(mean, pm)
                    nc.scalar.activation(std_t, pv, AF.Exp, scale=0.5)
                nc.sync.dma_start(out=stg[64:128, 6, :], in_=std_t)
                t1 = tmp.tile([64, NT], BF16, tag="t1", name="t1")
                nc.vector.tensor_mul(t1, std_t[:, :], ep[:, :])
                nc.vector.tensor_add(stg[0:64, 7, :], mean, t1[:, :])

            def stD(t):
                s = S[t]
                tin, stg = s["tin"], s["stg"]
                hg = tin[:, 0:4, :]
                gi = stg[0:80, 7, :]
                tzr_g = act.tile([128, 8, NT], BF16, tag="tzr_g",
                                 name="tzr_g")
                hg8 = s.get("hg8")
                for half in range(4):
                    pz = p2.tile([128, 2, NT], F32, tag="p2", name="pz_g")
                    for mi in range(2):
                        m = 2 * half + mi
                        pairs = [(Wgi[:, 0, ts(m, 128)], gi)]
                        if KNOBS["fp8_gen_h"]:
                            pairs += [(Wgzr[:, 2 * c:2 * c + 2, ts(m, 128)],
                                       hg8[:, 2 * c:2 * c + 2, :], "dr")
                                      for c in range(2)]
                        else:
                            pairs += [(Wgzr[:, k, ts(m, 128)], hg[:, k, :])
                                      for k in range(4)]
                        mm_group(pz[:, mi, :], pairs)
                    activ(tzr_g[:, 2 * half:2 * half + 2, :], pz, AF.Tanh,
                          scale=0.5, bias_tile=Bgzr, bias_c=2 * half, nch=2)
                    if half >= 2:
                        hh = half - 2
                        b_rg = tmp.tile([128, 2, NT], BF16, tag="b_rg",
                                        name="b_rg")
                        nc.vector.tensor_scalar_add(
                            b_rg, tzr_g[:, 2 * half:2 * half + 2, :], 1.0)
                        if hh == 0:
                            rh_g = tmp.tile(
                                [128, 4, NT],
                                FP8 if KNOBS["fp8_gen_rh"] else BF16,
                                tag="rh_g", name="rh_g")
                            s["rh_g"] = rh_g
                        ev(KNOBS["gp_rh_g"]).tensor_mul(
                            s["rh_g"][:, 2 * hh:2 * hh + 2, :],
                            b_rg[:, :, :], hg[:, 2 * hh:2 * hh + 2, :])
                s["tzr_g"] = tzr_g

            def stE(t):
                s = S[t]
                tin, stg = s["tin"], s["stg"]
                hg = tin[:, 0:4, :]
                gi = stg[0:80, 7, :]
                rh_g = s["rh_g"]
                ng = act.tile([128, 4, NT], BF16, tag="ng", name="ng")
                ngscale = (1.0 / WGN_SCALE) if KNOBS["fp8_gen_rh"] else 1.0
                for half in range(2):
                    pnv = p2.tile([128, 2, NT], F32, tag="p2", name="pn_g")
                    for mi in range(2):
                        m = 2 * half + mi
                        pairs = [(Wgi[:, 0, ts(8 + m, 128)], gi)]
                        if KNOBS["fp8_gen_rh"]:
                            pairs += [(Wgn[:, 2 * c:2 * c + 2, ts(m, 128)],
                                       rh_g[:, 2 * c:2 * c + 2, :], "dr")
                                      for c in range(2)]
                        else:
                            pairs += [(Wgn[:, k, ts(m, 128)], rh_g[:, k, :])
                                      for k in range(4)]
                        mm_group(pnv[:, mi, :], pairs)
                    activ(ng[:, 2 * half:2 * half + 2, :], pnv, AF.Tanh,
                          scale=ngscale, bias_tile=Bgn, bias_c=2 * half,
                          nch=2)
                a_zg = tmp.tile([128, 4, NT], BF16, tag="a_zg", name="a_zg")
                nc.vector.tensor_scalar(a_zg, s["tzr_g"][:, 0:4, :], 0.5, 0.5,
                                        op0=ALU.mult, op1=ALU.add)
                d_g = tmp.tile([128, 4, NT], BF16, tag="d_g", name="d_g")
                ev(KNOBS["gp_d_g"]).tensor_sub(d_g, hg, ng[:, :, :])
                nc.vector.tensor_mul(d_g, a_zg[:, :, :], d_g[:, :, :])
                hg2 = stg[:, 0:4, :]
                nc.vector.tensor_add(hg2, ng[:, :, :], d_g[:, :, :])
                nc.vector.tensor_scalar(hg2, hg2, -CLIP, CLIP,
                                        op0=ALU.max, op1=ALU.min)

            def stF(t):
                j = t % NTILES
                jc = ts(j, NT)
                s = S[t]
                stg = s["stg"]
                hg2 = stg[:, 0:4, :]
                pf = p1.tile([128, NT], F32, tag="pf1", name="pf", bufs=2)
                mm_group(pf, [(Wfac[:, k, :], hg2[:, k, :])
                              for k in range(4)])
                fc_t = tmp.tile([128, NT], BF16, tag="fc_t", name="fc_t")
                if KNOBS["fc_on_act"]:
                    nc.scalar.copy(fc_t, pf[:, :])
                else:
                    nc.vector.tensor_copy(fc_t, pf[:, :])
                nc.sync.dma_start(
                    out=outT[0:896, jc].rearrange("(c p) n -> p c n", p=128),
                    in_=stg[:, 0:7, :])
                nc.sync.dma_start(out=outT[896:976, jc],
                                  in_=stg[0:80, 7, :])
                nc.sync.dma_start(out=outT[976:H_DIM, jc], in_=fc_t)
                del S[t]

            total = NTILES * repeat
            for t in range(total):
                if t == 0:
                    stA(0)
                    stB(0)
                    stC(0)
                if t + 1 < total:
                    stA(t + 1)
                stD(t)
                if t + 1 < total:
                    stB(t + 1)
                if KNOBS["f_before_e"] and t >= 1:
                    stF(t - 1)
                stE(t)
                if not KNOBS["f_before_e"] and t >= 1:
                    stF(t - 1)
                if t + 1 < total:
                    stC(t + 1)
            stF(total - 1)

    nc.compile()
    return nc


# ---------------------------------------------------------------------------
# host-side runner (cached per process)
# ---------------------------------------------------------------------------
_CACHE = {}


def _get_runner(has_bias):
    key = has_bias
    if key not in _CACHE:
        nc = build_program(has_bias)
        _CACHE[key] = _make_runner(nc)
    return _CACHE[key]


def _make_runner(nc):
    import jax
    from jax.sharding import Mesh, PartitionSpec, NamedSharding
    from jax.experimental.shard_map import shard_map
    from concourse.bass2jax import (_bass_exec_p, install_neuronx_cc_hook,
                                    partition_id_tensor)

    install_neuronx_cc_hook()
    partition_name = (nc.partition_id_tensor.name
                      if nc.partition_id_tensor else None)
    in_names, out_names, out_avals, zero_outs = [], [], [], []
    for alloc in nc.m.functions[0].allocations:
        if not isinstance(alloc, mybir.MemoryLocationSet):
            continue
        name = alloc.memorylocations[0].name
        if alloc.kind == "ExternalInput":
            if name != partition_name:
                in_names.append(name)
        elif alloc.kind == "ExternalOutput":
            shape = tuple(alloc.tensor_shape)
            dtype = mybir.dt.np(alloc.dtype)
            out_names.append(name)
            out_avals.append(jax.core.ShapedArray(shape, dtype))
            zero_outs.append(np.zeros(shape, dtype))
    all_in = in_names + out_names
    if partition_name is not None:
        all_in.append(partition_name)

    def _body(*args):
        operands = list(args)
        if partition_name is not None:
            operands.append(partition_id_tensor())
        return tuple(_bass_exec_p.bind(
            *operands, out_avals=tuple(out_avals), in_names=tuple(all_in),
            out_names=tuple(out_names),
            lowering_input_output_aliases=(),
            sim_require_finite=True, sim_require_nnan=True, nc=nc))

    devices = jax.devices()[:N_CORES]
    mesh = Mesh(np.asarray(devices), ("core",))
    nin = len(in_names)
    fn = jax.jit(
        shard_map(_body, mesh=mesh,
                  in_specs=(PartitionSpec("core"),) * (nin + len(out_names)),
                  out_specs=(PartitionSpec("core"),) * len(out_names),
                  check_rep=False),
        keep_unused=True)
    sharding = NamedSharding(mesh, PartitionSpec("core"))

    class R:
        pass

    r = R()
    r.jax = jax
    r.fn = fn
    r.sharding = sharding
    r.in_names = in_names
    r.out_names = out_names
    r.out_avals = out_avals
    r.zero_outs = zero_outs
    return r


def _prep_inputs(input, h_0, eps, gen_w_ih, gen_w_hh, con_w_ih, con_w_hh,
                 co_w, fac_w, biases):
    import ml_dtypes
    BF = ml_dtypes.bfloat16
    f = np.float32
    input = np.asarray(input, f)
    h_0 = np.asarray(h_0, f)
    eps = np.asarray(eps, f)
    gen_w_ih = np.asarray(gen_w_ih, f)
    gen_w_hh = np.asarray(gen_w_hh, f)
    con_w_ih = np.asarray(con_w_ih, f)
    con_w_hh = np.asarray(con_w_hh, f)
    co_w = np.asarray(co_w, f)
    fac_w = np.asarray(fac_w, f)

    import ml_dtypes as _md
    E4 = _md.float8_e4m3
    norm = np.maximum(np.linalg.norm(fac_w, axis=1, keepdims=True), 1e-12)
    fac_wn = fac_w / norm

    # packed input rows: [hg(512); hc(256); fa(128); ci(256)] = 1152
    hg = h_0[:, :GEN]
    hc = h_0[:, GEN:GEN + CON]
    fa = h_0[:, GEN + CON + 3 * CO + EXT:]
    ci = input[:, :2 * CIE]
    ext = input[:, 2 * CIE:]
    packed = np.concatenate([hg, hc, fa, ci], axis=1).astype(BF)  # [B, 1152]

    per_core = {"inT": [], "xextT": [], "epsT": []}
    h8_parts = []
    if not KNOBS["h8_from_host"]:
        pass
    elif KNOBS["fp8_gen_h"]:
        h8_parts.append(hg.astype(BF).astype(E4))
    if KNOBS["h8_from_host"] and KNOBS["fp8_con_h"]:
        h8_parts.append(hc.astype(BF).astype(E4))
    if h8_parts:
        per_core["h8T"] = []
        h8_all = np.concatenate(h8_parts, axis=1)
    ext_b = ext.astype(BF)
    eps_b = eps.astype(BF)
    for c in range(N_CORES):
        rows = slice(c * B_CORE, (c + 1) * B_CORE)
        per_core["inT"].append(np.ascontiguousarray(packed[rows].T))
        per_core["xextT"].append(np.ascontiguousarray(ext_b[rows].T))
        per_core["epsT"].append(np.ascontiguousarray(eps_b[rows].T))
        if h8_parts:
            per_core["h8T"].append(np.ascontiguousarray(h8_all[rows].T))

    wgi = gen_w_ih.T.copy()  # [80, 1536]
    if KNOBS["fp8_gen_rh"]:
        wgi[:, 1024:] *= WGN_SCALE  # exact in bf16 (power of 2)
    wczr = con_w_hh[:2 * CON].T
    weights = {
        "Wci": np.ascontiguousarray(con_w_ih.T.astype(BF)),
        "Wczr": np.ascontiguousarray(
            wczr.astype(E4) if KNOBS["fp8_con_h"] else wczr.astype(BF)),
        "Wco": np.ascontiguousarray(co_w.T.astype(BF)),
        "Wgi": np.ascontiguousarray(wgi.astype(BF)),
        "Wfac": np.ascontiguousarray(fac_wn.T.astype(BF)),
    }
    wcn = (0.5 * con_w_hh[2 * CON:]).T
    weights["Wcn"] = np.ascontiguousarray(
        wcn.astype(E4) if KNOBS["fp8_con_rh"] else wcn.astype(BF))
    wgzr = gen_w_hh[:2 * GEN].T
    weights["Wgzr"] = np.ascontiguousarray(
        wgzr.astype(E4) if KNOBS["fp8_gen_h"] else wgzr.astype(BF))
    wgn = (0.5 * gen_w_hh[2 * GEN:]).T
    weights["Wgn"] = np.ascontiguousarray(
        (WGN_SCALE * wgn).astype(E4) if KNOBS["fp8_gen_rh"]
        else wgn.astype(BF))

    gen_b_ih, gen_b_hh, con_b_ih, con_b_hh, co_b = [
        np.asarray(b, f) for b in biases]
    has_bias = any(np.any(b) for b in (gen_b_ih, gen_b_hh, con_b_ih,
                                       con_b_hh, co_b))
    if has_bias:
        bc = con_b_ih + con_b_hh
        bg = gen_b_ih + gen_b_hh
        # tanh(0.5*(pre+b)) -> act bias = 0.5*b for z/r; n: bias = b
        weights["Bczr"] = np.ascontiguousarray(
            (0.5 * bc[:512]).reshape(4, 128).T.astype(f))
        weights["Bcn"] = np.ascontiguousarray(
            bc[512:].reshape(2, 128).T.astype(f))
        weights["Bgzr"] = np.ascontiguousarray(
            (0.5 * bg[:1024]).reshape(8, 128).T.astype(f))
        weights["Bgn"] = np.ascontiguousarray(
            bg[1024:].reshape(4, 128).T.astype(f))
        # co: mean bias = b[:64], std bias = 0.5*b[64:] (inside exp)
        weights["Bco"] = np.ascontiguousarray(
            np.concatenate([co_b[:64], 0.5 * co_b[64:]]).reshape(128, 1)
            .astype(f))
    return per_core, weights, has_bias


def kernel(input, h_0, eps, gen_w_ih, gen_b_ih, gen_w_hh, gen_b_hh,
           con_w_ih, con_b_ih, con_w_hh, con_b_hh, co_w, co_b, fac_w):
    per_core, weights, has_bias = _prep_inputs(
        input, h_0, eps, gen_w_ih, gen_w_hh, con_w_ih, con_w_hh, co_w, fac_w,
        (gen_b_ih, gen_b_hh, con_b_ih, con_b_hh, co_b))

    r = _get_runner(has_bias)
    jax = r.jax

    args = []
    for name in r.in_names:
        if name in per_core:
            a = np.concatenate(per_core[name], axis=0)
        else:
            a = np.concatenate([weights[name]] * N_CORES, axis=0)
        args.append(jax.device_put(a, r.sharding))
    for z in r.zero_outs:
        args.append(jax.device_put(
            np.zeros((N_CORES * z.shape[0], *z.shape[1:]), z.dtype),
            r.sharding))

    outs = jax.block_until_ready(r.fn(*args))
    outT_all = np.asarray(outs[r.out_names.index("outT")])
    outT_all = outT_all.reshape(N_CORES, H_DIM, B_CORE)

    out = np.empty((BATCH, H_DIM), np.float32)
    for c in range(N_CORES):
        out[c * B_CORE:(c + 1) * B_CORE] = \
            outT_all[c].T.astype(np.float32)
    return out


# revision 15
# speedup vs baseline: 2.6331x; 2.6331x over previous
"""Trainium2 Bass kernel for nn_DecoderCell (LFADS-style decoder cell).

Strategy: pure data parallel over 8 NeuronCores (batch 32768 -> 4096/core).
v2: bf16 end-to-end.
 - Host packs inputs feature-major as ONE bf16 tensor per core
   ([hg; hc; fa; ci] = 1152 rows) so each batch tile needs a single big
   input DMA; eps/xext ride separate small tensors. Output is staged in
   one bf16 SBUF tile per batch tile (compute ops write final values
   directly into their outT row positions) and stored with one DMA.
 - All matmuls bf16 x bf16 -> f32 PSUM (1 cyc/row, same rate as f32r,
   half the DMA bytes).
 - Sigmoid is computed as 0.5 + 0.5*tanh(x/2) so every activation
   (tanh/exp/identity/copy) lives in ONE act table set -> zero act-table
   switches. The 0.5 factors fold into tensor_scalar ops / W_n weights.
 - GRU update h' = n + a*(h-n) with a = 0.5 + 0.5*t_z via DVE
   tensor_scalar (4x bf16 mode) + tensor_tensor (2x), split across
   Vector and GPSIMD engines.
"""

import sys

sys.path.insert(0, "/opt/trn_rl_repo")

import numpy as np

import concourse.bacc as bacc
import concourse.tile as tile
import concourse.mybir as mybir

N_CORES = 8
BATCH = 32768
B_CORE = BATCH // N_CORES  # 4096
NT = 512                   # batch tile (free dim per matmul / PSUM bank)
NTILES = B_CORE // NT      # 8

GEN, CON, CO, FAC, CIE, EXT = 512, 256, 64, 128, 128, 16
X_DIM = 2 * CIE + EXT      # 272
H_DIM = GEN + CON + 3 * CO + EXT + FAC  # 1104
CLIP = 5.0

IN_ROWS = GEN + CON + FAC + 2 * CIE     # 1152 = 9 chunks of 128
IN_CH = IN_ROWS // 128                  # 9
OUT_CH = 8                              # stg: hg2,hc2,mean/std,gi+xext

F32 = mybir.dt.float32
BF16 = mybir.dt.bfloat16
FP8 = mybir.dt.float8e4
DRM = mybir.MatmulPerfMode.DoubleRow
WGN_SCALE = 32.0
AF = mybir.ActivationFunctionType
ALU = mybir.AluOpType

KNOBS = {
    "in_bufs": 3, "stg_bufs": 3, "eps_bufs": 2,
    "act_bufs": 2, "tmp_bufs": 2,
    "p2_bufs": 2, "p1_bufs": 1,
    # engine assignment: True -> gpsimd(Pool), False -> vector(DVE)
    "gp_rh_c": False, "gp_d_c": False,
    "gp_rh_g": False, "gp_d_g": False,
    # fp8 DoubleRow matmuls (precision-validated against the reference)
    "fp8_gen_h": True, "fp8_gen_rh": True, "fp8_con_rh": False,
    "fp8_con_h": True,
    "h8_from_host": True,
    "fc_on_act": True, "f_before_e": False,
}


def ts(i, s):
    return slice(i * s, (i + 1) * s)


def build_program(has_bias: bool, repeat: int = 1):
    nc = bacc.Bacc("TRN2", target_bir_lowering=False, debug=False,
                   num_devices=N_CORES)

    inT = nc.dram_tensor("inT", [IN_ROWS, B_CORE], BF16, kind="ExternalInput")
    xextT = nc.dram_tensor("xextT", [EXT, B_CORE], BF16, kind="ExternalInput")
    epsT = nc.dram_tensor("epsT", [CO, B_CORE], BF16, kind="ExternalInput")
    Wci_d = nc.dram_tensor("Wci", [384, 768], BF16, kind="ExternalInput")
    Wczr_d = nc.dram_tensor("Wczr", [256, 512],
                            FP8 if KNOBS["fp8_con_h"] else BF16,
                            kind="ExternalInput")
    Wcn_d = nc.dram_tensor("Wcn", [256, 256],
                           FP8 if KNOBS["fp8_con_rh"] else BF16,
                           kind="ExternalInput")
    Wco_d = nc.dram_tensor("Wco", [256, 128], BF16, kind="ExternalInput")
    Wgi_d = nc.dram_tensor("Wgi", [80, 1536], BF16, kind="ExternalInput")
    Wgzr_d = nc.dram_tensor("Wgzr", [512, 1024],
                            FP8 if KNOBS["fp8_gen_h"] else BF16,
                            kind="ExternalInput")
    Wgn_d = nc.dram_tensor("Wgn", [512, 512],
                           FP8 if KNOBS["fp8_gen_rh"] else BF16,
                           kind="ExternalInput")
    Wfac_d = nc.dram_tensor("Wfac", [512, 128], BF16, kind="ExternalInput")
    H8_ROWS = (GEN if KNOBS["fp8_gen_h"] else 0) + \
        (CON if KNOBS["fp8_con_h"] else 0)
    if H8_ROWS and KNOBS["h8_from_host"]:
        h8T = nc.dram_tensor("h8T", [H8_ROWS, B_CORE], FP8,
                             kind="ExternalInput")
    if has_bias:
        # per-partition bias columns, pre-scaled on host where needed
        Bczr_d = nc.dram_tensor("Bczr", [128, 4], F32, kind="ExternalInput")
        Bcn_d = nc.dram_tensor("Bcn", [128, 2], F32, kind="ExternalInput")
        Bgzr_d = nc.dram_tensor("Bgzr", [128, 8], F32, kind="ExternalInput")
        Bgn_d = nc.dram_tensor("Bgn", [128, 4], F32, kind="ExternalInput")
        Bco_d = nc.dram_tensor("Bco", [128, 1], F32, kind="ExternalInput")

    outT = nc.dram_tensor("outT", [H_DIM, B_CORE], BF16,
                          kind="ExternalOutput")

    # staging row map: hg2 rows 0..511 stg[:,0:4]; hc2 rows 512..767
    # stg[:,4:6]; mean rows 768..831 stg[0:64,6]; std rows 832..895
    # stg[64:128,6] (filled by SB->SB DMA from a base-0 tile, since all
    # TensorTensor math must be partition-base aligned); gi rows 896..975
    # stg[0:80,7] ([gi_co(64); xext(16)]). fc (rows 976..1103) bypasses
    # staging and is stored from its own base-0 tile.

    with tile.TileContext(nc) as tc:
        with (
            tc.tile_pool(name="w", bufs=1) as wp,
            tc.tile_pool(name="io", bufs=KNOBS["in_bufs"]) as io,
            tc.tile_pool(name="stg", bufs=KNOBS["stg_bufs"]) as stgp,
            tc.tile_pool(name="eps", bufs=KNOBS["eps_bufs"]) as epsp,
            tc.tile_pool(name="act", bufs=KNOBS["act_bufs"]) as act,
            tc.tile_pool(name="tmp", bufs=KNOBS["tmp_bufs"]) as tmp,
            tc.tile_pool(name="p2", bufs=KNOBS["p2_bufs"], space="PSUM") as p2,
            tc.tile_pool(name="p1", bufs=KNOBS["p1_bufs"], space="PSUM") as p1,
        ):
            # ---- load weights once (feature-major [k, m]) ----
            def wload(dram, K, M, tag):
                kc = (K + 127) // 128
                dt = dram.dtype
                if K % 128 == 0:
                    t = wp.tile([128, kc, M], dt, tag=tag, name=tag)
                    nc.sync.dma_start(
                        out=t,
                        in_=dram[:, :].rearrange("(c p) m -> p c m", p=128))
                else:
                    assert kc == 1
                    t = wp.tile([K, 1, M], dt, tag=tag, name=tag)
                    nc.sync.dma_start(out=t, in_=dram[:, :].unsqueeze(1))
                return t

            Wci = wload(Wci_d, 384, 768, "Wci")
            Wczr = wload(Wczr_d, 256, 512, "Wczr")
            Wcn = wload(Wcn_d, 256, 256, "Wcn")
            Wco = wload(Wco_d, 256, 128, "Wco")
            Wgi = wload(Wgi_d, 80, 1536, "Wgi")
            Wgzr = wload(Wgzr_d, 512, 1024, "Wgzr")
            Wgn = wload(Wgn_d, 512, 512, "Wgn")
            Wfac = wload(Wfac_d, 512, 128, "Wfac")
            if has_bias:
                Bczr = wp.tile([128, 4], F32, tag="Bczr", name="Bczr")
                nc.sync.dma_start(out=Bczr, in_=Bczr_d[:, :])
                Bcn = wp.tile([128, 2], F32, tag="Bcn", name="Bcn")
                nc.sync.dma_start(out=Bcn, in_=Bcn_d[:, :])
                Bgzr = wp.tile([128, 8], F32, tag="Bgzr", name="Bgzr")
                nc.sync.dma_start(out=Bgzr, in_=Bgzr_d[:, :])
                Bgn = wp.tile([128, 4], F32, tag="Bgn", name="Bgn")
                nc.sync.dma_start(out=Bgn, in_=Bgn_d[:, :])
                Bco = wp.tile([128, 1], F32, tag="Bco", name="Bco")
                nc.sync.dma_start(out=Bco, in_=Bco_d[:, :])

            def mm_group(psum_out, pairs):
                n = len(pairs)
                for i, p in enumerate(pairs):
                    lh, rh = p[0], p[1]
                    pm_ = DRM if len(p) > 2 else None
                    nc.tensor.matmul(psum_out, lh, rh,
                                     start=(i == 0), stop=(i == n - 1),
                                     perf_mode=pm_)

            if not has_bias:
                Bczr = Bcn = Bgzr = Bgn = Bco = None

            def activ(dst, src, func, scale=1.0, bias_tile=None, bias_c=0,
                      nch=1):
                if bias_tile is not None:
                    for c in range(nch):
                        nc.scalar.activation(
                            dst[:, c, :] if nch > 1 else dst,
                            src[:, c, :] if nch > 1 else src,
                            func, scale=scale,
                            bias=bias_tile[:, bias_c + c:bias_c + c + 1])
                else:
                    nc.scalar.activation(dst, src, func, scale=scale)

            def eng(gp):
                return nc.gpsimd if gp else nc.vector

            # ------------- software-pipelined stages -------------
            # Per tile t: A=load+con-zr, B=con-n+h'c, C=co+gi,
            # D=gen-zr, E=gen-n+h'g, F=fac+store.
            # Steady emission: A[t+1] D[t] B[t+1] E[t] F[t-1] C[t+1]
            # keeps the in-order PE queue stall-free: every dependent
            # stage has another tile's matmuls emitted between it and
            # its producer.
            S = {}

            def ev(gp):
                return nc.gpsimd if gp else nc.vector

            def stA(t):
                j = t % NTILES
                jc = ts(j, NT)
                s = S[t] = {}
                tin = io.tile([128, IN_CH, NT], BF16, tag="tin", name="tin")
                nc.sync.dma_start(
                    out=tin,
                    in_=inT[:, jc].rearrange("(c p) n -> p c n", p=128))
                stg = stgp.tile([128, OUT_CH, NT], BF16, tag="stg",
                                name="stg")
                nc.sync.dma_start(out=stg[64:80, 7, :], in_=xextT[:, jc])
                ep = epsp.tile([64, NT], BF16, tag="ep", name="ep")
                nc.sync.dma_start(out=ep, in_=epsT[:, jc])
                s["tin"], s["stg"], s["ep"] = tin, stg, ep
                if H8_ROWS:
                    nch8 = H8_ROWS // 128
                    h8 = io.tile([128, nch8, NT], FP8, tag="h8", name="h8")
                    if KNOBS["h8_from_host"]:
                        nc.sync.dma_start(
                            out=h8,
                            in_=h8T[:, jc].rearrange("(c p) n -> p c n",
                                                     p=128))
                    else:
                        if KNOBS["fp8_gen_h"]:
                            nc.scalar.copy(h8[:, 0:4, :], tin[:, 0:4, :])
                        if KNOBS["fp8_con_h"]:
                            nc.scalar.copy(h8[:, nch8 - 2:nch8, :],
                                           tin[:, 4:6, :])
                    if KNOBS["fp8_gen_h"]:
                        s["hg8"] = h8[:, 0:4, :]
                    if KNOBS["fp8_con_h"]:
                        s["hc8"] = h8[:, nch8 - 2:nch8, :]
                hc = tin[:, 4:6, :]
                x_rhs = [tin[:, 7, :], tin[:, 8, :], tin[:, 6, :]]
                s["x_rhs"] = x_rhs

                tzr_c = act.tile([128, 4, NT], BF16, tag="tzr_c",
                                 name="tzr_c")
                for half in range(2):
                    pz = p2.tile([128, 2, NT], F32, tag="p2", name="pz_c")
                    for mi in range(2):
                        m = 2 * half + mi
                        pairs = [(Wci[:, k, ts(m, 128)], x_rhs[k])
                                 for k in range(3)]
                        if KNOBS["fp8_con_h"]:
                            pairs += [(Wczr[:, 0:2, ts(m, 128)],
                                       s["hc8"][:, 0:2, :], "dr")]
                        else:
                            pairs += [(Wczr[:, k, ts(m, 128)], hc[:, k, :])
                                      for k in range(2)]
                        mm_group(pz[:, mi, :], pairs)
                    activ(tzr_c[:, 2 * half:2 * half + 2, :], pz, AF.Tanh,
                          scale=0.5, bias_tile=Bczr, bias_c=2 * half, nch=2)
                s["tzr_c"] = tzr_c
                b_rc = tmp.tile([128, 2, NT], BF16, tag="b_rc", name="b_rc")
                nc.vector.tensor_scalar_add(b_rc, tzr_c[:, 2:4, :], 1.0)
                rh_c = tmp.tile([128, 2, NT],
                                FP8 if KNOBS["fp8_con_rh"] else BF16,
                                tag="rh_c", name="rh_c")
                ev(KNOBS["gp_rh_c"]).tensor_mul(rh_c, b_rc[:, :, :],
                                                hc[:, :, :])
                s["rh_c"] = rh_c

            def stB(t):
                s = S[t]
                tin, stg = s["tin"], s["stg"]
                hc, rh_c = tin[:, 4:6, :], s["rh_c"]
                pn = p2.tile([128, 2, NT], F32, tag="p2", name="pn_c")
                for mi in range(2):
                    pairs = [(Wci[:, k, ts(4 + mi, 128)], s["x_rhs"][k])
                             for k in range(3)]
                    if KNOBS["fp8_con_rh"]:
                        pairs += [(Wcn[:, 0:2, ts(mi, 128)],
                                   rh_c[:, 0:2, :], "dr")]
                    else:
                        pairs += [(Wcn[:, k, ts(mi, 128)], rh_c[:, k, :])
                                  for k in range(2)]
                    mm_group(pn[:, mi, :], pairs)
                n_c = act.tile([128, 2, NT], BF16, tag="n_c", name="n_c")
                activ(n_c, pn, AF.Tanh, bias_tile=Bcn, nch=2)
                a_zc = tmp.tile([128, 2, NT], BF16, tag="a_zc", name="a_zc")
                nc.vector.tensor_scalar(a_zc, s["tzr_c"][:, 0:2, :], 0.5, 0.5,
                                        op0=ALU.mult, op1=ALU.add)
                d_c = tmp.tile([128, 2, NT], BF16, tag="d_c", name="d_c")
                ev(KNOBS["gp_d_c"]).tensor_sub(d_c, hc, n_c[:, :, :])
                nc.vector.tensor_mul(d_c, a_zc[:, :, :], d_c[:, :, :])
                hc2 = stg[:, 4:6, :]
                nc.vector.tensor_add(hc2, n_c[:, :, :], d_c[:, :, :])
                nc.vector.tensor_scalar(hc2, hc2, -CLIP, CLIP,
                                        op0=ALU.max, op1=ALU.min)

            def stC(t):
                s = S[t]
                stg, ep = s["stg"], s["ep"]
                hc2 = stg[:, 4:6, :]
                pm = p1.tile([64, NT], F32, tag="pm", name="pm")
                mm_group(pm, [(Wco[:, k, 0:64], hc2[:, k, :])
                              for k in range(2)])
                pv = p1.tile([64, NT], F32, tag="pv", name="pv")
                mm_group(pv, [(Wco[:, k, 64:128], hc2[:, k, :])
                              for k in range(2)])
                mean = stg[0:64, 6, :]
                std_t = tmp.tile([64, NT], BF16, tag="std_t", name="std_t")
                if has_bias:
                    nc.scalar.activation(mean, pm, AF.Identity,
                                         bias=Bco[0:64, 0:1])
                    nc.scalar.activation(std_t, pv, AF.Exp,
                                         scale=0.5, bias=Bco[64:128, 0:1])
                else:
                    nc.scalar.copy(mean, pm)
                    nc.scalar.activation(std_t, pv, AF.Exp, scale=0.5)
                nc.sync.dma_start(out=stg[64:128, 6, :], in_=std_t)
                t1 = tmp.tile([64, NT], BF16, tag="t1", name="t1")
                nc.vector.tensor_mul(t1, std_t[:, :], ep[:, :])
                nc.vector.tensor_add(stg[0:64, 7, :], mean, t1[:, :])

            def stD(t):
                s = S[t]
                tin, stg = s["tin"], s["stg"]
                hg = tin[:, 0:4, :]
                gi = stg[0:80, 7, :]
                tzr_g = act.tile([128, 8, NT], BF16, tag="tzr_g",
                                 name="tzr_g")
                hg8 = s.get("hg8")
                for half in range(4):
                    pz = p2.tile([128, 2, NT], F32, tag="p2", name="pz_g")
                    for mi in range(2):
                        m = 2 * half + mi
                        pairs = [(Wgi[:, 0, ts(m, 128)], gi)]
                        if KNOBS["fp8_gen_h"]:
                            pairs += [(Wgzr[:, 2 * c:2 * c + 2, ts(m, 128)],
                                       hg8[:, 2 * c:2 * c + 2, :], "dr")
                                      for c in range(2)]
                        else:
                            pairs += [(Wgzr[:, k, ts(m, 128)], hg[:, k, :])
                                      for k in range(4)]
                        mm_group(pz[:, mi, :], pairs)
                    activ(tzr_g[:, 2 * half:2 * half + 2, :], pz, AF.Tanh,
                          scale=0.5, bias_tile=Bgzr, bias_c=2 * half, nch=2)
                    if half >= 2:
                        hh = half - 2
                        b_rg = tmp.tile([128, 2, NT], BF16, tag="b_rg",
                                        name="b_rg")
                        nc.vector.tensor_scalar_add(
                            b_rg, tzr_g[:, 2 * half:2 * half + 2, :], 1.0)
                        if hh == 0:
                            rh_g = tmp.tile(
                                [128, 4, NT],
                                FP8 if KNOBS["fp8_gen_rh"] else BF16,
                                tag="rh_g", name="rh_g")
                            s["rh_g"] = rh_g
                        ev(KNOBS["gp_rh_g"]).tensor_mul(
                            s["rh_g"][:, 2 * hh:2 * hh + 2, :],
                            b_rg[:, :, :], hg[:, 2 * hh:2 * hh + 2, :])
                s["tzr_g"] = tzr_g

            def stE(t):
                s = S[t]
                tin, stg = s["tin"], s["stg"]
                hg = tin[:, 0:4, :]
                gi = stg[0:80, 7, :]
                rh_g = s["rh_g"]
                ng = act.tile([128, 4, NT], BF16, tag="ng", name="ng")
                ngscale = (1.0 / WGN_SCALE) if KNOBS["fp8_gen_rh"] else 1.0
                for half in range(2):
                    pnv = p2.tile([128, 2, NT], F32, tag="p2", name="pn_g")
                    for mi in range(2):
                        m = 2 * half + mi
                        pairs = [(Wgi[:, 0, ts(8 + m, 128)], gi)]
                        if KNOBS["fp8_gen_rh"]:
                            pairs += [(Wgn[:, 2 * c:2 * c + 2, ts(m, 128)],
                                       rh_g[:, 2 * c:2 * c + 2, :], "dr")
                                      for c in range(2)]
                        else:
                            pairs += [(Wgn[:, k, ts(m, 128)], rh_g[:, k, :])
                                      for k in range(4)]
                        mm_group(pnv[:, mi, :], pairs)
                    activ(ng[:, 2 * half:2 * half + 2, :], pnv, AF.Tanh,
                          scale=ngscale, bias_tile=Bgn, bias_c=2 * half,
                          nch=2)
                a_zg = tmp.tile([128, 4, NT], BF16, tag="a_zg", name="a_zg")
                nc.vector.tensor_scalar(a_zg, s["tzr_g"][:, 0:4, :], 0.5, 0.5,
                                        op0=ALU.mult, op1=ALU.add)
                d_g = tmp.tile([128, 4, NT], BF16, tag="d_g", name="d_g")
                ev(KNOBS["gp_d_g"]).tensor_sub(d_g, hg, ng[:, :, :])
                nc.vector.tensor_mul(d_g, a_zg[:, :, :], d_g[:, :, :])
                hg2 = stg[:, 0:4, :]
                nc.vector.tensor_add(hg2, ng[:, :, :], d_g[:, :, :])
                nc.vector.tensor_scalar(hg2, hg2, -CLIP, CLIP,
                                        op0=ALU.max, op1=ALU.min)

            def stF(t):
                j = t % NTILES
                jc = ts(j, NT)
                s = S[t]
                stg = s["stg"]
                hg2 = stg[:, 0:4, :]
                pf = p1.tile([128, NT], F32, tag="pf1", name="pf", bufs=2)
                mm_group(pf, [(Wfac[:, k, :], hg2[:, k, :])
                              for k in range(4)])
                fc_t = tmp.tile([128, NT], BF16, tag="fc_t", name="fc_t")
                if KNOBS["fc_on_act"]:
                    nc.scalar.copy(fc_t, pf[:, :])
                else:
                    nc.vector.tensor_copy(fc_t, pf[:, :])
                nc.sync.dma_start(
                    out=outT[0:896, jc].rearrange("(c p) n -> p c n", p=128),
                    in_=stg[:, 0:7, :])
                nc.sync.dma_start(out=outT[896:976, jc],
                                  in_=stg[0:80, 7, :])
                nc.sync.dma_start(out=outT[976:H_DIM, jc], in_=fc_t)
                del S[t]

            total = NTILES * repeat
            for t in range(total):
                if t == 0:
                    stA(0)
                    stB(0)
                    stC(0)
                if t + 1 < total:
                    stA(t + 1)
                stD(t)
                if t + 1 < total:
                    stB(t + 1)
                if KNOBS["f_before_e"] and t >= 1:
                    stF(t - 1)
                stE(t)
                if not KNOBS["f_before_e"] and t >= 1:
                    stF(t - 1)
                if t + 1 < total:
                    stC(t + 1)
            stF(total - 1)

    nc.compile()
    return nc


# ---------------------------------------------------------------------------
# host-side runner (cached per process)
# ---------------------------------------------------------------------------
_CACHE = {}


def _get_runner(has_bias):
    key = has_bias
    if key not in _CACHE:
        nc = build_program(has_bias)
        _CACHE[key] = _make_runner(nc)
    return _CACHE[key]


def _make_runner(nc):
    import jax
    from jax.sharding import Mesh, PartitionSpec, NamedSharding
    from jax.experimental.shard_map import shard_map
    from concourse.bass2jax import (_bass_exec_p, install_neuronx_cc_hook,
                                    partition_id_tensor)

    install_neuronx_cc_hook()
    partition_name = (nc.partition_id_tensor.name
                      if nc.partition_id_tensor else None)
    in_names, out_names, out_avals, zero_outs = [], [], [], []
    for alloc in nc.m.functions[0].allocations:
        if not isinstance(alloc, mybir.MemoryLocationSet):
            continue
        name = alloc.memorylocations[0].name
        if alloc.kind == "ExternalInput":
            if name != partition_name:
                in_names.append(name)
        elif alloc.kind == "ExternalOutput":
            shape = tuple(alloc.tensor_shape)
            dtype = mybir.dt.np(alloc.dtype)
            out_names.append(name)
            out_avals.append(jax.core.ShapedArray(shape, dtype))
            zero_outs.append(np.zeros(shape, dtype))
    all_in = in_names + out_names
    if partition_name is not None:
        all_in.append(partition_name)

    def _body(*args):
        operands = list(args)
        if partition_name is not None:
            operands.append(partition_id_tensor())
        return tuple(_bass_exec_p.bind(
            *operands, out_avals=tuple(out_avals), in_names=tuple(all_in),
            out_names=tuple(out_names),
            lowering_input_output_aliases=(),
            sim_require_finite=True, sim_require_nnan=True, nc=nc))

    devices = jax.devices()[:N_CORES]
    mesh = Mesh(np.asarray(devices), ("core",))
    nin = len(in_names)
    fn = jax.jit(
        shard_map(_body, mesh=mesh,
                  in_specs=(PartitionSpec("core"),) * (nin + len(out_names)),
                  out_specs=(PartitionSpec("core"),) * len(out_names),
                  check_rep=False),
        keep_unused=True)
    sharding = NamedSharding(mesh, PartitionSpec("core"))

    class R:
        pass

    r = R()
    r.jax = jax
    r.fn = fn
    r.sharding = sharding
    r.in_names = in_names
    r.out_names = out_names
    r.out_avals = out_avals
    r.zero_outs = zero_outs
    return r


def _prep_inputs(input, h_0, eps, gen_w_ih, gen_w_hh, con_w_ih, con_w_hh,
                 co_w, fac_w, biases):
    import ml_dtypes
    BF = ml_dtypes.bfloat16
    f = np.float32
    input = np.asarray(input, f)
    h_0 = np.asarray(h_0, f)
    eps = np.asarray(eps, f)
    gen_w_ih = np.asarray(gen_w_ih, f)
    gen_w_hh = np.asarray(gen_w_hh, f)
    con_w_ih = np.asarray(con_w_ih, f)
    con_w_hh = np.asarray(con_w_hh, f)
    co_w = np.asarray(co_w, f)
    fac_w = np.asarray(fac_w, f)

    import ml_dtypes as _md
    E4 = _md.float8_e4m3
    norm = np.maximum(np.linalg.norm(fac_w, axis=1, keepdims=True), 1e-12)
    fac_wn = fac_w / norm

    # packed input rows: [hg(512); hc(256); fa(128); ci(256)] = 1152
    hg = h_0[:, :GEN]
    hc = h_0[:, GEN:GEN + CON]
    fa = h_0[:, GEN + CON + 3 * CO + EXT:]
    ci = input[:, :2 * CIE]
    ext = input[:, 2 * CIE:]
    packed = np.concatenate([hg, hc, fa, ci], axis=1).astype(BF)  # [B, 1152]

    per_core = {"inT": [], "xextT": [], "epsT": []}
    h8_parts = []
    if not KNOBS["h8_from_host"]:
        pass
    elif KNOBS["fp8_gen_h"]:
        h8_parts.append(hg.astype(BF).astype(E4))
    if KNOBS["h8_from_host"] and KNOBS["fp8_con_h"]:
        h8_parts.append(hc.astype(BF).astype(E4))
    if h8_parts:
        per_core["h8T"] = []
        h8_all = np.concatenate(h8_parts, axis=1)
    ext_b = ext.astype(BF)
    eps_b = eps.astype(BF)
    for c in range(N_CORES):
        rows = slice(c * B_CORE, (c + 1) * B_CORE)
        per_core["inT"].append(np.ascontiguousarray(packed[rows].T))
        per_core["xextT"].append(np.ascontiguousarray(ext_b[rows].T))
        per_core["epsT"].append(np.ascontiguousarray(eps_b[rows].T))
        if h8_parts:
            per_core["h8T"].append(np.ascontiguousarray(h8_all[rows].T))

    wgi = gen_w_ih.T.copy()  # [80, 1536]
    if KNOBS["fp8_gen_rh"]:
        wgi[:, 1024:] *= WGN_SCALE  # exact in bf16 (power of 2)
    wczr = con_w_hh[:2 * CON].T
    weights = {
        "Wci": np.ascontiguousarray(con_w_ih.T.astype(BF)),
        "Wczr": np.ascontiguousarray(
            wczr.astype(E4) if KNOBS["fp8_con_h"] else wczr.astype(BF)),
        "Wco": np.ascontiguousarray(co_w.T.astype(BF)),
        "Wgi": np.ascontiguousarray(wgi.astype(BF)),
        "Wfac": np.ascontiguousarray(fac_wn.T.astype(BF)),
    }
    wcn = (0.5 * con_w_hh[2 * CON:]).T
    weights["Wcn"] = np.ascontiguousarray(
        wcn.astype(E4) if KNOBS["fp8_con_rh"] else wcn.astype(BF))
    wgzr = gen_w_hh[:2 * GEN].T
    weights["Wgzr"] = np.ascontiguousarray(
        wgzr.astype(E4) if KNOBS["fp8_gen_h"] else wgzr.astype(BF))
    wgn = (0.5 * gen_w_hh[2 * GEN:]).T
    weights["Wgn"] = np.ascontiguousarray(
        (WGN_SCALE * wgn).astype(E4) if KNOBS["fp8_gen_rh"]
        else wgn.astype(BF))

    gen_b_ih, gen_b_hh, con_b_ih, con_b_hh, co_b = [
        np.asarray(b, f) for b in biases]
    has_bias = any(np.any(b) for b in (gen_b_ih, gen_b_hh, con_b_ih,
                                       con_b_hh, co_b))
    if has_bias:
        bc = con_b_ih + con_b_hh
        bg = gen_b_ih + gen_b_hh
        # tanh(0.5*(pre+b)) -> act bias = 0.5*b for z/r; n: bias = b
        weights["Bczr"] = np.ascontiguousarray(
            (0.5 * bc[:512]).reshape(4, 128).T.astype(f))
        weights["Bcn"] = np.ascontiguousarray(
            bc[512:].reshape(2, 128).T.astype(f))
        weights["Bgzr"] = np.ascontiguousarray(
            (0.5 * bg[:1024]).reshape(8, 128).T.astype(f))
        weights["Bgn"] = np.ascontiguousarray(
            bg[1024:].reshape(4, 128).T.astype(f))
        # co: mean bias = b[:64], std bias = 0.5*b[64:] (inside exp)
        weights["Bco"] = np.ascontiguousarray(
            np.concatenate([co_b[:64], 0.5 * co_b[64:]]).reshape(128, 1)
            .astype(f))
    return per_core, weights, has_bias


def kernel(input, h_0, eps, gen_w_ih, gen_b_ih, gen_w_hh, gen_b_hh,
           con_w_ih, con_b_ih, con_w_hh, con_b_hh, co_w, co_b, fac_w):
    per_core, weights, has_bias = _prep_inputs(
        input, h_0, eps, gen_w_ih, gen_w_hh, con_w_ih, con_w_hh, co_w, fac_w,
        (gen_b_ih, gen_b_hh, con_b_ih, con_b_hh, co_b))

    r = _get_runner(has_bias)
    jax = r.jax

    args = []
    for name in r.in_names:
        if name in per_core:
            a = np.concatenate(per_core[name], axis=0)
        else:
            a = np.concatenate([weights[name]] * N_CORES, axis=0)
        args.append(jax.device_put(a, r.sharding))
    for z in r.zero_outs:
        args.append(jax.device_put(
            np.zeros((N_CORES * z.shape[0], *z.shape[1:]), z.dtype),
            r.sharding))

    outs = jax.block_until_ready(r.fn(*args))
    outT_all = np.asarray(outs[r.out_names.index("outT")])
    outT_all = outT_all.reshape(N_CORES, H_DIM, B_CORE)

    out = np.empty((BATCH, H_DIM), np.float32)
    for c in range(N_CORES):
        out[c * B_CORE:(c + 1) * B_CORE] = \
            outT_all[c].T.astype(np.float32)
    return out
